# revision 5
# baseline (speedup 1.0000x reference)
"""RSNA loss kernel for Trainium2, SPMD across 8 NeuronCores.

Strategy (data-parallel over batch, host-side dtype compression):
  - Shard B=128 exams -> 16 per core. Per exam, view [8192, 10] as
    [128 part, 64 j, 10 c] (partition p holds l in [64p, 64p+64)).
  - The seq_len mask over (p, j) is rank-2: mask = a[p]s[j] + b[p]t[j];
    masked channel sums are TensorE matmuls contracting p with a/b
    columns; the tiny j-fold with s/t happens on HOST from raw PSUM
    partials.
  - Host packs everything as fp8e4m3 (x32 scaled):
      blob8 [128, 16, 1216]: pred ch1-9 || label ch0-9 per exam ->
        DoubleRow fp8 matmuls process 2 exams per pass.
      blob0 [128, 16, 128]: dense p0 || y0 -> Scalar Ln with
        scale=1/32 dequant, DVE bce, per-quad a/b matmuls.
  - PE warmup: a chain of dummy DoubleRow matmuls on a memset tile
    pre-ramps the PE DVFS (0.65 -> 2.4 GHz takes ~3us continuous) so
    the real accumulation chain runs hot and pipelined.
  - 3 DMA rings (sync/scalar HWDGE + gpsimd SWDGE) stream chunks
    sized to finish together; matmul program order matches arrival.
  - Raw [32,1216]+[8,1024] f32 partials DMA'd out; host folds with
    s/t and combines the loss in f64.
"""
import numpy as np
from contextlib import ExitStack

import concourse.bass as bass
import concourse.bacc as bacc
import concourse.tile as tile
from concourse import mybir
from concourse.bass_utils import run_bass_kernel_spmd

N_CORES = 8
B, L, C = 128, 8192, 10
EPC = B // N_CORES          # exams per core = 16
JP = 64                     # l's per partition per exam
NPART = 128
SCALE = 32.0                # fp8 scaling (keeps (0.01,0.99) in normal range)
SCOLS = 1216                # 576 pred ch1-9 + 640 label ch0-9
NPAIR = EPC // 2            # 8 DoubleRow pairs
NQUAD = EPC // 4            # 4 bce quads
WARMUP = 30                 # PE pre-ramp matmuls

IMAGE_WEIGHT = 0.0736196319
EXAM_WEIGHTS = np.array([0.0736196319, 0.09202453988, 0.1042944785, 0.1042944785,
                         0.1877300613, 0.06257668712, 0.06257668712, 0.2346625767,
                         0.0782208589], dtype=np.float64)

f8 = mybir.dt.float8e4
bf16 = mybir.dt.bfloat16
f32 = mybir.dt.float32
F8NP = mybir.dt.np(f8)
BF16NP = mybir.dt.np(bf16)

# chunk schedule: (ring, what) in issue order; "e",lo,hi = blob8 exams
SYNC_CHUNKS = [("lhst",), ("e", 0, 2), ("e", 2, 6)]
SCALAR_CHUNKS = [("b0",), ("e", 6, 12)]
GPSIMD_CHUNKS = [("abq",), ("e", 12, 16)]
PAIR_ORDER = [0, 1, 2, 3, 4, 5, 6, 7]

_NC_CACHE = {}


def build_nc():
    nc = bacc.Bacc(trn_type="TRN2")
    blob8 = nc.declare_dram_parameter("blob8", [NPART, EPC, SCOLS], f8,
                                      isOutput=False)
    blob0 = nc.declare_dram_parameter("blob0", [NPART, EPC, 3 * JP], f8,
                                      isOutput=False)
    lhst8 = nc.declare_dram_parameter("lhst8", [NPART, NPAIR, 2, 32], f8,
                                      isOutput=False)
    abq = nc.declare_dram_parameter("abq", [NPART, NQUAD, 2, 32], bf16,
                                    isOutput=False)
    outS = nc.declare_dram_parameter("outS", [32, SCOLS], f32, isOutput=True)
    outB = nc.declare_dram_parameter("outB", [8, EPC * JP], f32, isOutput=True)

    DR = mybir.MatmulPerfMode.DoubleRow
    ACT = mybir.ActivationFunctionType

    with tile.TileContext(nc) as tc, ExitStack() as ctx:
        sb = ctx.enter_context(tc.tile_pool(name="sb", bufs=1))
        ps = ctx.enter_context(tc.tile_pool(name="ps", bufs=1, space="PSUM"))

        T8 = sb.tile([NPART, EPC, SCOLS], f8, tag="T8")
        T0 = sb.tile([NPART, EPC, 3 * JP], f8, tag="T0")
        t_lhst8 = sb.tile([NPART, NPAIR, 2, 32], f8, tag="lhst8")
        t_abq = sb.tile([NPART, NQUAD, 2, 32], bf16, tag="abq")

        # PE warmup: memset a dummy tile, then hammer small DR matmuls
        WU = sb.tile([NPART, 256], f8, tag="WU")
        nc.vector.memset(WU, 0.0)
        PW = ps.tile([32, 128], f32, tag="PW")
        wu_l = WU[:, 0:64].rearrange("p (k m) -> p k m", k=2)
        wu_r = WU.rearrange("p (k c) -> p k c", k=2)
        for i in range(WARMUP):
            nc.tensor.matmul(PW, wu_l, wu_r, perf_mode=DR,
                             start=True, stop=True)

        # --- DMA issue (all rings, in schedule order) ---
        def issue(engine, chunks):
            for ch in chunks:
                if ch[0] == "lhst":
                    engine.dma_start(out=t_lhst8, in_=lhst8[:, :, :, :])
                elif ch[0] == "abq":
                    engine.dma_start(out=t_abq, in_=abq[:, :, :, :])
                elif ch[0] == "b0":
                    engine.dma_start(out=T0, in_=blob0[:, :, :])
                else:
                    _, lo, hi = ch
                    engine.dma_start(out=T8[:, lo:hi], in_=blob8[:, lo:hi])

        issue(nc.sync, SYNC_CHUNKS)
        issue(nc.scalar, SCALAR_CHUNKS)
        issue(nc.gpsimd, GPSIMD_CHUNKS)

        # --- PSUM accumulators ---
        P0 = ps.tile([32, 512], f32, tag="P0")
        P1 = ps.tile([32, 512], f32, tag="P1")
        P2 = ps.tile([32, 192], f32, tag="P2")
        PB = [ps.tile([8, 4 * JP], f32, tag=f"PB{q}", name=f"PB{q}")
              for q in range(NQUAD)]

        # --- channel-sum DoubleRow matmuls, ordered by expected arrival ---
        V8 = T8.rearrange("p (t k) c -> p t k c", k=2)  # [128, 8, 2, 1216]
        for i, t in enumerate(PAIR_ORDER):
            st = dict(start=(i == 0), stop=(i == len(PAIR_ORDER) - 1))
            lhsT = t_lhst8[:, t]
            rhs = V8[:, t]
            nc.tensor.matmul(P0, lhsT, rhs[:, :, 0:512], perf_mode=DR, **st)
            nc.tensor.matmul(P1, lhsT, rhs[:, :, 512:1024], perf_mode=DR, **st)
            nc.tensor.matmul(P2, lhsT, rhs[:, :, 1024:1216], perf_mode=DR, **st)

        # --- image BCE path (inputs are 32*p0, 32*y0 in fp8) ---
        LP = sb.tile([NPART, EPC, JP], bf16, tag="LP")
        LQ = sb.tile([NPART, EPC, JP], bf16, tag="LQ")
        Y0B = sb.tile([NPART, EPC, JP], bf16, tag="Y0B")
        BCE = sb.tile([NPART, EPC, JP], bf16, tag="BCE")
        for h in range(2):
            sl = slice(8 * h, 8 * h + 8)
            p0 = T0[:, sl, 0:JP]
            q0 = T0[:, sl, JP:2 * JP]
            y0 = T0[:, sl, 2 * JP:3 * JP]
            nc.scalar.activation(out=LP[:, sl], in_=p0, func=ACT.Ln,
                                 scale=1.0 / SCALE)
            nc.scalar.activation(out=LQ[:, sl], in_=q0, func=ACT.Ln,
                                 scale=1.0 / SCALE)
            nc.scalar.mul(Y0B[:, sl], y0, 1.0 / SCALE)
            nc.vector.tensor_sub(BCE[:, sl], LP[:, sl], LQ[:, sl])
            nc.vector.tensor_mul(BCE[:, sl], BCE[:, sl], Y0B[:, sl])
            nc.vector.tensor_add(BCE[:, sl], BCE[:, sl], LQ[:, sl])
            for qq in range(2):
                q = 2 * h + qq
                nc.tensor.matmul(PB[q], t_abq[:, q, 0, 0:8], BCE[:, 4 * q:4 * q + 4],
                                 start=True, stop=True)

        # --- PSUM -> SBUF -> DRAM ---
        SB_ = sb.tile([8, EPC * JP], f32, tag="SB_")
        for q in range(NQUAD):
            eng = nc.vector.tensor_copy if q % 2 == 0 else nc.scalar.copy
            eng(SB_[:, 4 * JP * q:4 * JP * (q + 1)], PB[q])
        nc.gpsimd.dma_start(out=outB[:, :], in_=SB_)
        S0 = sb.tile([32, 512], f32, tag="S0")
        S1 = sb.tile([32, 512], f32, tag="S1")
        S2 = sb.tile([32, 192], f32, tag="S2")
        nc.scalar.copy(S0, P0)
        nc.vector.tensor_copy(S1, P1)
        nc.vector.tensor_copy(S2, P2)
        nc.sync.dma_start(out=outS[:, 0:512], in_=S0)
        nc.scalar.dma_start(out=outS[:, 512:1024], in_=S1)
        nc.gpsimd.dma_start(out=outS[:, 1024:1216], in_=S2)
    nc.finalize()
    return nc


def _mask_ab_st(lens):
    """Per-exam a[p], b[p], s[j], t[j] float masks from seq_lens."""
    lens = np.asarray(lens, np.int64)
    P, r = np.divmod(lens, JP)
    p_idx = np.arange(NPART)
    j_idx = np.arange(JP)
    a = (p_idx[None, :] <= P[:, None]).astype(np.float64)   # [E, 128]
    b = (p_idx[None, :] < P[:, None]).astype(np.float64)
    s = (j_idx[None, :] < r[:, None]).astype(np.float64)    # [E, 64]
    t = 1.0 - s
    return a, b, s, t


def _mask_tensors(lens):
    a, b, s, t = _mask_ab_st(lens)
    lhst8 = np.zeros((NPART, NPAIR, 2, 32), np.float32)
    abq = np.zeros((NPART, NQUAD, 2, 32), np.float32)
    for e in range(EPC):
        pr, k = divmod(e, 2)
        lhst8[:, pr, k, 2 * e] = a[e]
        lhst8[:, pr, k, 2 * e + 1] = b[e]
        q, i = divmod(e, 4)
        abq[:, q, 0, 2 * i] = a[e]
        abq[:, q, 0, 2 * i + 1] = b[e]
    return lhst8.astype(F8NP), abq.astype(BF16NP)


def make_in_maps(pred, label, seq_lens):
    pred = np.asarray(pred)
    label = np.asarray(label)
    seq_lens = np.asarray(seq_lens)
    # [B, 8192, 10] -> [cores, 128p, 16e, 64j, 10c]
    pv = pred.reshape(N_CORES, EPC, NPART, JP, C).transpose(0, 2, 1, 3, 4)
    lv = label.reshape(N_CORES, EPC, NPART, JP, C).transpose(0, 2, 1, 3, 4)
    blob8 = np.empty((N_CORES, NPART, EPC, SCOLS), np.float32)
    blob8[..., 0:576] = (pv[..., 1:] * SCALE).reshape(
        N_CORES, NPART, EPC, 576)
    blob8[..., 576:1216] = (lv * SCALE).reshape(N_CORES, NPART, EPC, 640)
    blob8 = blob8.astype(F8NP)
    blob0 = np.empty((N_CORES, NPART, EPC, 3 * JP), np.float32)
    blob0[..., 0:JP] = pv[..., 0] * SCALE
    blob0[..., JP:2 * JP] = (1.0 - pv[..., 0]) * SCALE
    blob0[..., 2 * JP:] = lv[..., 0] * SCALE
    blob0 = blob0.astype(F8NP)

    in_maps = []
    for i in range(N_CORES):
        lhst8, abq = _mask_tensors(seq_lens[i * EPC:(i + 1) * EPC])
        in_maps.append({
            "blob8": np.ascontiguousarray(blob8[i]),
            "blob0": np.ascontiguousarray(blob0[i]),
            "lhst8": lhst8, "abq": abq,
        })
    return in_maps


def finish(outs, seq_lens):
    """Host combine from per-core raw partials ([32,1216], [8,1024])."""
    w = EXAM_WEIGHTS
    exam_loss = 0.0
    image_loss = 0.0
    tw_img = 0.0
    for i in range(N_CORES):
        lens = seq_lens[i * EPC:(i + 1) * EPC].astype(np.float64)
        a, b, s, t = _mask_ab_st(seq_lens[i * EPC:(i + 1) * EPC])
        S = outs[i]["outS"].astype(np.float64)      # [32, 1216]
        Sa = S[0::2].reshape(EPC, SCOLS)             # a-parts per exam
        Sb = S[1::2].reshape(EPC, SCOLS)
        # pred ch1-9: cols 0:576 as [64 j, 9 c]; label: 576:1216 as [64, 10]
        pa = Sa[:, 0:576].reshape(EPC, JP, 9)
        pb = Sb[:, 0:576].reshape(EPC, JP, 9)
        la = Sa[:, 576:1216].reshape(EPC, JP, C)
        lb = Sb[:, 576:1216].reshape(EPC, JP, C)
        pm_raw = np.einsum('ejc,ej->ec', pa, s) + np.einsum('ejc,ej->ec', pb, t)
        lm_raw = np.einsum('ejc,ej->ec', la, s) + np.einsum('ejc,ej->ec', lb, t)
        pm = np.clip(pm_raw / SCALE / lens[:, None], 2**-20, 1 - 2**-20)
        ym = lm_raw[:, 1:] / SCALE / lens[:, None]
        y0m = lm_raw[:, 0] / SCALE / lens
        exam_bce = -(ym * np.log(pm) + (1.0 - ym) * np.log(1.0 - pm))
        exam_loss += float(np.sum(exam_bce * w[None, :]))
        # bce partials: outB [8 rows, 1024]: quad q, exam e=4q+i ->
        # rows 2i/2i+1, cols 256q + 64i + j
        OB = outs[i]["outB"].astype(np.float64).reshape(8, NQUAD, 4, JP)
        ba = np.empty((EPC, JP))
        bb = np.empty((EPC, JP))
        for e in range(EPC):
            q, ii = divmod(e, 4)
            ba[e] = OB[2 * ii, q, ii]
            bb[e] = OB[2 * ii + 1, q, ii]
        bce_sum = np.einsum('ej,ej->e', ba, s) + np.einsum('ej,ej->e', bb, t)
        imgw = IMAGE_WEIGHT * y0m
        image_loss += float(np.sum(-bce_sum * imgw))
        tw_img += float(np.sum(imgw * lens))
    total_weights = B * float(np.sum(w)) + tw_img
    return np.float32((exam_loss + image_loss) / total_weights)


def kernel(pred, label, seq_lens):
    if "nc" not in _NC_CACHE:
        _NC_CACHE["nc"] = build_nc()
    nc = _NC_CACHE["nc"]
    in_maps = make_in_maps(pred, label, seq_lens)
    res = run_bass_kernel_spmd(nc, in_maps, core_ids=list(range(N_CORES)))
    return finish(res.results, np.asarray(seq_lens))


if __name__ == "__main__":
    rng = np.random.default_rng(0)
    pred = (rng.random((B, L, C), np.float32) * 0.98 + 0.01).astype(np.float32)
    label = (rng.random((B, L, C), np.float32) * 0.98 + 0.01).astype(np.float32)
    seq_lens = rng.integers(1, L + 1, size=(B,)).astype(np.int32)
    got = kernel(pred=pred, label=label, seq_lens=seq_lens)
    print("kernel:", got)


# revision 7
# speedup vs baseline: 1.0769x; 1.0769x over previous
"""RSNA loss kernel for Trainium2, SPMD across 8 NeuronCores.

Strategy (data-parallel over batch, host-side dtype compression):
  - Shard B=128 exams -> 16 per core. Per exam, view [8192, 10] as
    [128 part, 64 j, 10 c] (partition p holds l in [64p, 64p+64)).
  - The seq_len mask over (p, j) is rank-2: mask = a[p]s[j] + b[p]t[j];
    masked channel sums are TensorE matmuls contracting p with a/b
    columns; the tiny j-fold with s/t happens on HOST from raw PSUM
    partials.
  - Host packs everything as fp8e4m3 (x32 scaled):
      blob8 [128, 16, 1216]: pred ch1-9 || label ch0-9 per exam ->
        DoubleRow fp8 matmuls process 2 exams per pass.
      blob0 [128, 16, 128]: dense p0 || y0 -> Scalar Ln with
        scale=1/32 dequant, DVE bce, per-quad a/b matmuls.
  - PE warmup: a chain of dummy DoubleRow matmuls on a memset tile
    pre-ramps the PE DVFS (0.65 -> 2.4 GHz takes ~3us continuous) so
    the real accumulation chain runs hot and pipelined.
  - 3 DMA rings (sync/scalar HWDGE + gpsimd SWDGE) stream chunks
    sized to finish together; matmul program order matches arrival.
  - Raw [32,1216]+[8,1024] f32 partials DMA'd out; host folds with
    s/t and combines the loss in f64.
"""
import numpy as np
from contextlib import ExitStack

import concourse.bass as bass
import concourse.bacc as bacc
import concourse.tile as tile
from concourse import mybir
from concourse.bass_utils import run_bass_kernel_spmd

N_CORES = 8
B, L, C = 128, 8192, 10
EPC = B // N_CORES          # exams per core = 16
JP = 64                     # l's per partition per exam
NPART = 128
SCALE = 32.0                # fp8 scaling (keeps (0.01,0.99) in normal range)
SCOLS = 1216                # 576 pred ch1-9 + 640 label ch0-9
NPAIR = EPC // 2            # 8 DoubleRow pairs
NQUAD = EPC // 4            # 4 bce quads
WARMUP = 30                 # PE pre-ramp matmuls

IMAGE_WEIGHT = 0.0736196319
EXAM_WEIGHTS = np.array([0.0736196319, 0.09202453988, 0.1042944785, 0.1042944785,
                         0.1877300613, 0.06257668712, 0.06257668712, 0.2346625767,
                         0.0782208589], dtype=np.float64)

f8 = mybir.dt.float8e4
bf16 = mybir.dt.bfloat16
f32 = mybir.dt.float32
F8NP = mybir.dt.np(f8)
BF16NP = mybir.dt.np(bf16)

# chunk schedule: (ring, what) in issue order; "e",lo,hi = blob8 exams
SYNC_CHUNKS = [("lhst",), ("e", 0, 2), ("e", 2, 6), ("e", 14, 16)]
SCALAR_CHUNKS = [("b0",), ("e", 6, 10), ("e", 10, 12)]
GPSIMD_CHUNKS = [("abq",), ("e", 12, 14)]
A_ORDER = [0, 1, 2, 3]
B_ORDER = [4, 6, 5, 7]
FILLERS = 2

_NC_CACHE = {}


def build_nc():
    nc = bacc.Bacc(trn_type="TRN2")
    blob8 = nc.declare_dram_parameter("blob8", [NPART, EPC, SCOLS], f8,
                                      isOutput=False)
    blob0 = nc.declare_dram_parameter("blob0", [NPART, EPC, 3 * JP], f8,
                                      isOutput=False)
    lhst8 = nc.declare_dram_parameter("lhst8", [NPART, NPAIR, 2, 32], f8,
                                      isOutput=False)
    abq = nc.declare_dram_parameter("abq", [NPART, 2, 64], bf16,
                                    isOutput=False)
    outS = nc.declare_dram_parameter("outS", [2, 32, SCOLS], f32, isOutput=True)
    outB = nc.declare_dram_parameter("outB", [16, EPC * JP], f32, isOutput=True)

    DR = mybir.MatmulPerfMode.DoubleRow
    ACT = mybir.ActivationFunctionType

    with tile.TileContext(nc) as tc, ExitStack() as ctx:
        sb = ctx.enter_context(tc.tile_pool(name="sb", bufs=1))
        ps = ctx.enter_context(tc.tile_pool(name="ps", bufs=1, space="PSUM"))

        T8 = sb.tile([NPART, EPC, SCOLS], f8, tag="T8")
        T0 = sb.tile([NPART, EPC, 3 * JP], f8, tag="T0")
        t_lhst8 = sb.tile([NPART, NPAIR, 2, 32], f8, tag="lhst8")
        t_abq = sb.tile([NPART, 2, 64], bf16, tag="abq")

        # PE warmup tile: zero weights/data; warmup + fillers accumulate
        # exact zeros into the live P0a/P0b chains (start flag on first)
        WU = sb.tile([NPART, 1024], f8, tag="WU")
        nc.vector.memset(WU, 0.0)
        wu_l = WU[:, 0:64].rearrange("p (k m) -> p k m", k=2)
        wu_r = WU.rearrange("p (k c) -> p k c", k=2)

        # --- DMA issue (all rings, in schedule order) ---
        def issue(engine, chunks):
            for ch in chunks:
                if ch[0] == "lhst":
                    engine.dma_start(out=t_lhst8, in_=lhst8[:, :, :, :])
                elif ch[0] == "abq":
                    engine.dma_start(out=t_abq, in_=abq[:, :, :])
                elif ch[0] == "b0":
                    engine.dma_start(out=T0, in_=blob0[:, :, :])
                else:
                    _, lo, hi = ch
                    engine.dma_start(out=T8[:, lo:hi], in_=blob8[:, lo:hi])

        issue(nc.sync, SYNC_CHUNKS)
        issue(nc.scalar, SCALAR_CHUNKS)
        issue(nc.gpsimd, GPSIMD_CHUNKS)

        # --- PSUM accumulators (split chains: a = pairs 0-3, b = 4-7) ---
        P0a = ps.tile([32, 512], f32, tag="P0a")
        P1a = ps.tile([32, 512], f32, tag="P1a")
        P2a = ps.tile([32, 192], f32, tag="P2a")
        P0b = ps.tile([32, 512], f32, tag="P0b")
        P1b = ps.tile([32, 512], f32, tag="P1b")
        P2b = ps.tile([32, 192], f32, tag="P2b")
        PBh = [ps.tile([16, 8 * JP], f32, tag=f"PBh{h}", name=f"PBh{h}")
               for h in range(2)]

        # --- channel-sum DoubleRow matmuls, ordered by expected arrival;
        # warmup + zero fillers keep the PE DVFS clock ramped ---
        V8 = T8.rearrange("p (t k) c -> p t k c", k=2)  # [128, 8, 2, 1216]

        def chain(P0x, P1x, P2x, order, warm):
            for i in range(warm):
                nc.tensor.matmul(P0x, wu_l, wu_r2, perf_mode=DR,
                                 start=(i == 0), stop=False)
            for i, t in enumerate(order):
                if i > 0:
                    for _ in range(FILLERS):
                        nc.tensor.matmul(P0x, wu_l, wu_r2, perf_mode=DR,
                                         start=False, stop=False)
                last = (i == len(order) - 1)
                lhsT = t_lhst8[:, t]
                rhs = V8[:, t]
                nc.tensor.matmul(P0x, lhsT, rhs[:, :, 0:512], perf_mode=DR,
                                 start=False, stop=last)
                nc.tensor.matmul(P1x, lhsT, rhs[:, :, 512:1024], perf_mode=DR,
                                 start=(i == 0), stop=last)
                nc.tensor.matmul(P2x, lhsT, rhs[:, :, 1024:1216], perf_mode=DR,
                                 start=(i == 0), stop=last)

        wu_r2 = WU.rearrange("p (k c) -> p k c", k=2)  # [128, 2, 512]
        chain(P0a, P1a, P2a, A_ORDER, WARMUP)
        chain(P0b, P1b, P2b, B_ORDER, 1)

        # --- image BCE path (inputs are 32*p0, 32*y0 in fp8) ---
        LP = sb.tile([NPART, EPC, JP], bf16, tag="LP")
        LQ = sb.tile([NPART, EPC, JP], bf16, tag="LQ")
        Y0B = sb.tile([NPART, EPC, JP], bf16, tag="Y0B")
        BCE = sb.tile([NPART, EPC, JP], bf16, tag="BCE")
        for h in range(2):
            sl = slice(8 * h, 8 * h + 8)
            p0 = T0[:, sl, 0:JP]
            q0 = T0[:, sl, JP:2 * JP]
            y0 = T0[:, sl, 2 * JP:3 * JP]
            nc.scalar.activation(out=LP[:, sl], in_=p0, func=ACT.Ln,
                                 scale=1.0 / SCALE)
            nc.scalar.activation(out=LQ[:, sl], in_=q0, func=ACT.Ln,
                                 scale=1.0 / SCALE)
            nc.scalar.mul(Y0B[:, sl], y0, 1.0 / SCALE)
            nc.vector.tensor_sub(BCE[:, sl], LP[:, sl], LQ[:, sl])
            nc.vector.tensor_mul(BCE[:, sl], BCE[:, sl], Y0B[:, sl])
            nc.vector.tensor_add(BCE[:, sl], BCE[:, sl], LQ[:, sl])
            nc.tensor.matmul(PBh[h], t_abq[:, h, 0:16], BCE[:, sl],
                             start=True, stop=True)

        # --- PSUM -> SBUF -> DRAM ---
        SB_ = sb.tile([16, EPC * JP], f32, tag="SB_")
        nc.vector.tensor_copy(SB_[:, 0:512], PBh[0])
        nc.scalar.copy(SB_[:, 512:1024], PBh[1])
        nc.gpsimd.dma_start(out=outB[:, :], in_=SB_)
        SA = sb.tile([32, SCOLS], f32, tag="SA")
        SBt = sb.tile([32, SCOLS], f32, tag="SBt")
        nc.scalar.copy(SA[:, 0:512], P0a)
        nc.vector.tensor_copy(SA[:, 512:1024], P1a)
        nc.vector.tensor_copy(SA[:, 1024:1216], P2a)
        nc.gpsimd.dma_start(out=outS[0], in_=SA)
        nc.scalar.copy(SBt[:, 0:512], P0b)
        nc.vector.tensor_copy(SBt[:, 512:1024], P1b)
        nc.vector.tensor_copy(SBt[:, 1024:1216], P2b)
        nc.sync.dma_start(out=outS[1], in_=SBt)
    nc.finalize()
    return nc


def _mask_ab_st(lens):
    """Per-exam a[p], b[p], s[j], t[j] float masks from seq_lens."""
    lens = np.asarray(lens, np.int64)
    P, r = np.divmod(lens, JP)
    p_idx = np.arange(NPART)
    j_idx = np.arange(JP)
    a = (p_idx[None, :] <= P[:, None]).astype(np.float64)   # [E, 128]
    b = (p_idx[None, :] < P[:, None]).astype(np.float64)
    s = (j_idx[None, :] < r[:, None]).astype(np.float64)    # [E, 64]
    t = 1.0 - s
    return a, b, s, t


def _mask_tensors(lens):
    a, b, s, t = _mask_ab_st(lens)
    lhst8 = np.zeros((NPART, NPAIR, 2, 32), np.float32)
    abq = np.zeros((NPART, 2, 64), np.float32)
    for e in range(EPC):
        pr, k = divmod(e, 2)
        lhst8[:, pr, k, 2 * e] = a[e]
        lhst8[:, pr, k, 2 * e + 1] = b[e]
        h, i = divmod(e, 8)
        abq[:, h, 2 * i] = a[e]
        abq[:, h, 2 * i + 1] = b[e]
    return lhst8.astype(F8NP), abq.astype(BF16NP)


def make_in_maps(pred, label, seq_lens):
    pred = np.asarray(pred)
    label = np.asarray(label)
    seq_lens = np.asarray(seq_lens)
    # [B, 8192, 10] -> [cores, 128p, 16e, 64j, 10c]
    pv = pred.reshape(N_CORES, EPC, NPART, JP, C).transpose(0, 2, 1, 3, 4)
    lv = label.reshape(N_CORES, EPC, NPART, JP, C).transpose(0, 2, 1, 3, 4)
    blob8 = np.empty((N_CORES, NPART, EPC, SCOLS), np.float32)
    blob8[..., 0:576] = (pv[..., 1:] * SCALE).reshape(
        N_CORES, NPART, EPC, 576)
    blob8[..., 576:1216] = (lv * SCALE).reshape(N_CORES, NPART, EPC, 640)
    blob8 = blob8.astype(F8NP)
    blob0 = np.empty((N_CORES, NPART, EPC, 3 * JP), np.float32)
    blob0[..., 0:JP] = pv[..., 0] * SCALE
    blob0[..., JP:2 * JP] = (1.0 - pv[..., 0]) * SCALE
    blob0[..., 2 * JP:] = lv[..., 0] * SCALE
    blob0 = blob0.astype(F8NP)

    in_maps = []
    for i in range(N_CORES):
        lhst8, abq = _mask_tensors(seq_lens[i * EPC:(i + 1) * EPC])
        in_maps.append({
            "blob8": np.ascontiguousarray(blob8[i]),
            "blob0": np.ascontiguousarray(blob0[i]),
            "lhst8": lhst8, "abq": abq,
        })
    return in_maps


def finish(outs, seq_lens):
    """Host combine from per-core raw partials ([32,1216], [8,1024])."""
    w = EXAM_WEIGHTS
    exam_loss = 0.0
    image_loss = 0.0
    tw_img = 0.0
    for i in range(N_CORES):
        lens = seq_lens[i * EPC:(i + 1) * EPC].astype(np.float64)
        a, b, s, t = _mask_ab_st(seq_lens[i * EPC:(i + 1) * EPC])
        S = outs[i]["outS"].astype(np.float64).sum(axis=0)  # [32, 1216]
        Sa = S[0::2].reshape(EPC, SCOLS)             # a-parts per exam
        Sb = S[1::2].reshape(EPC, SCOLS)
        # pred ch1-9: cols 0:576 as [64 j, 9 c]; label: 576:1216 as [64, 10]
        pa = Sa[:, 0:576].reshape(EPC, JP, 9)
        pb = Sb[:, 0:576].reshape(EPC, JP, 9)
        la = Sa[:, 576:1216].reshape(EPC, JP, C)
        lb = Sb[:, 576:1216].reshape(EPC, JP, C)
        pm_raw = np.einsum('ejc,ej->ec', pa, s) + np.einsum('ejc,ej->ec', pb, t)
        lm_raw = np.einsum('ejc,ej->ec', la, s) + np.einsum('ejc,ej->ec', lb, t)
        pm = np.clip(pm_raw / SCALE / lens[:, None], 2**-20, 1 - 2**-20)
        ym = lm_raw[:, 1:] / SCALE / lens[:, None]
        y0m = lm_raw[:, 0] / SCALE / lens
        exam_bce = -(ym * np.log(pm) + (1.0 - ym) * np.log(1.0 - pm))
        exam_loss += float(np.sum(exam_bce * w[None, :]))
        # bce partials: outB [8 rows, 1024]: quad q, exam e=4q+i ->
        # rows 2i/2i+1, cols 256q + 64i + j
        OB = outs[i]["outB"].astype(np.float64).reshape(16, 2, 8, JP)
        ba = np.empty((EPC, JP))
        bb = np.empty((EPC, JP))
        for e in range(EPC):
            h, ii = divmod(e, 8)
            ba[e] = OB[2 * ii, h, ii]
            bb[e] = OB[2 * ii + 1, h, ii]
        bce_sum = np.einsum('ej,ej->e', ba, s) + np.einsum('ej,ej->e', bb, t)
        imgw = IMAGE_WEIGHT * y0m
        image_loss += float(np.sum(-bce_sum * imgw))
        tw_img += float(np.sum(imgw * lens))
    total_weights = B * float(np.sum(w)) + tw_img
    return np.float32((exam_loss + image_loss) / total_weights)


def kernel(pred, label, seq_lens):
    if "nc" not in _NC_CACHE:
        _NC_CACHE["nc"] = build_nc()
    nc = _NC_CACHE["nc"]
    in_maps = make_in_maps(pred, label, seq_lens)
    res = run_bass_kernel_spmd(nc, in_maps, core_ids=list(range(N_CORES)))
    return finish(res.results, np.asarray(seq_lens))


if __name__ == "__main__":
    rng = np.random.default_rng(0)
    pred = (rng.random((B, L, C), np.float32) * 0.98 + 0.01).astype(np.float32)
    label = (rng.random((B, L, C), np.float32) * 0.98 + 0.01).astype(np.float32)
    seq_lens = rng.integers(1, L + 1, size=(B,)).astype(np.int32)
    got = kernel(pred=pred, label=label, seq_lens=seq_lens)
    print("kernel:", got)
